# revision 42
# baseline (speedup 1.0000x reference)
"""Multi-head attention (B=2, D=1024, L=2048, H=16) on 8 TRN2 NeuronCores.

Sharding (per spec hint): tensor-parallel over heads x data-parallel over
batch.  Core c handles batch c//4 and head group g=c%4 (4 heads as 2
"pairs" of 2 heads stacked on 64-partition halves).  Host sums the 4
bf16 partial outputs per batch (row-parallel W_O) in f32 -- no on-device
collective.

Per core:
  - Q/K projected into pair layout (head dims on partitions); V projected
    directly into transposed layout VT (Lk x dh) with a ones column per
    head so A@V also emits the softmax denominator.
  - Scores ST[k,q]: the two heads' K=64 matmuls are issued back-to-back
    at partition bases 0/64; they run concurrently on disjoint PE row
    groups (~2x, verified in trace).
  - exp runs on the scalar/ACT engine (the only exp engine): 128 x
    (128,1024) activations ~= 142us is the kernel's pacing floor.  All
    other PE work (pair-1 Q/K projections, output projection) is emitted
    as fillers INSIDE attention units so it executes in the PE's
    ACT-bound gaps instead of serializing.
  - Normalization per (pair, qb): denominator rows -> DVE reciprocal ->
    K=2 selector matmul broadcast -> one multiply.  The matmul+multiply
    (norm_b) is emitted one unit later so the PE queue never waits on
    the (slow, ~3.3us) DVE reciprocal.

All matmuls bf16 (f32 PSUM accumulate); softmax stats f32.
"""

import sys
import types

import numpy as np
import ml_dtypes


def _install_axon_hooks_shim():
    try:
        import antenv.axon_hooks  # noqa: F401
        return
    except ImportError:
        pass
    try:
        import antenv
    except ImportError:
        return
    mod = types.ModuleType("antenv.axon_hooks")
    mod._hook = None
    mod.set_axon_ntff_profile_hook = lambda h: setattr(mod, "_hook", h)
    mod.get_axon_ntff_profile_hook = lambda: mod._hook
    sys.modules["antenv.axon_hooks"] = mod
    antenv.axon_hooks = mod
    try:
        from trn_agent_boot.trn_boot import _ntff_profile_via_ctypes

        h = _ntff_profile_via_ctypes("/opt/axon/libaxon_pjrt.so")
        if h is not None:
            mod._hook = h
    except Exception:
        pass


_install_axon_hooks_shim()

import concourse.bass as bass  # noqa: E402
import concourse.mybir as mybir  # noqa: E402
import concourse.tile as tile  # noqa: E402
from concourse import bacc  # noqa: E402
from concourse.bass_utils import run_bass_kernel_spmd  # noqa: E402
from concourse.tile_rust import add_dep_helper  # noqa: E402

BF16 = mybir.dt.bfloat16
F32 = mybir.dt.float32
AF = mybir.ActivationFunctionType

B, D, L, H = 2, 1024, 2048, 16
DH = D // H            # 64
P = 128
SCALE = 1.0 / np.sqrt(np.float32(DH))

DC = D // P            # 8 contraction chunks over D
LT = L // P            # 16 Lk tiles
HV = DH + 1            # V^T per-head width incl. ones column
NPAIR = 2              # head pairs per core (4 heads)
NQB = 4                # query blocks of 512
QB = L // NQB          # 512


def build():
    nc = bacc.Bacc(None, target_bir_lowering=False, debug=False)

    x = nc.dram_tensor("x", [D, L], BF16, kind="ExternalInput")
    wq = nc.dram_tensor("wq", [D, NPAIR * P], BF16, kind="ExternalInput")
    wk = nc.dram_tensor("wk", [D, NPAIR * P], BF16, kind="ExternalInput")
    wv = nc.dram_tensor("wv", [D, NPAIR * P], BF16, kind="ExternalInput")
    wo = nc.dram_tensor("wo", [NPAIR * P, D], BF16, kind="ExternalInput")
    sel2 = nc.dram_tensor("sel2", [DH + 1, P], BF16, kind="ExternalInput")
    out = nc.dram_tensor("out", [D, L], BF16, kind="ExternalOutput")

    xr = x[:].rearrange("(o p) l -> p o l", p=P)          # (128, 8, 2048)
    wqr = wq[:].rearrange("(o p) m -> p o m", p=P)        # (128, 8, 256)
    wkr = wk[:].rearrange("(o p) m -> p o m", p=P)
    wvr = wv[:].rearrange("(o p) m -> p o m", p=P)
    wor = wo[:].rearrange("(j p) o -> p j o", p=P)        # (128, 2, 1024)
    outr = out[:].rearrange("(o p) l -> p o l", p=P)      # (128, 8, 2048)

    with tile.TileContext(nc) as tc:
        with (
            tc.tile_pool(name="consts", bufs=1) as consts,
            tc.tile_pool(name="res", bufs=1) as res,
            tc.tile_pool(name="exp", bufs=3) as epool,
            tc.tile_pool(name="norm", bufs=2) as npool,
            tc.tile_pool(name="outp", bufs=3) as opool,
            tc.tile_pool(name="ps_s", bufs=2, space="PSUM") as ps_s,
            tc.tile_pool(name="ps_c", bufs=3, space="PSUM") as ps_c,
            tc.tile_pool(name="ps_f", bufs=1, space="PSUM") as ps_f,
        ):
            # DMA wave sequencing: the DMA engines round-robin among ALL
            # in-flight transfers, so if everything is enqueued at once the
            # startup-critical wq/x0/x1 only land when ~everything lands
            # (~24us).  Explicit wave dependencies keep at most 3-4
            # transfers in flight so the early waves get full bandwidth.
            sel2_sb = consts.tile([DH + 1, P], BF16)
            wq_sb = res.tile([P, DC, NPAIR * P], BF16)
            wk_sb = res.tile([P, DC, NPAIR * P], BF16)
            wv_sb = res.tile([P, DC, NPAIR * P], BF16)
            wo_sb = res.tile([P, NPAIR, D], BF16)
            xb = res.tile([P, DC, L], BF16)

            nc.sync.dma_start(out=sel2_sb[:], in_=sel2[:])
            w1 = [
                nc.sync.dma_start(out=wq_sb[:], in_=wqr),
                nc.gpsimd.dma_start(out=xb[:, 0, :], in_=xr[:, 0, :]),
                nc.scalar.dma_start(out=wv_sb[:], in_=wvr),
            ]
            w2 = [
                nc.sync.dma_start(out=xb[:, 1, :], in_=xr[:, 1, :]),
                nc.gpsimd.dma_start(out=xb[:, 2, :], in_=xr[:, 2, :]),
                nc.scalar.dma_start(out=xb[:, 3, :], in_=xr[:, 3, :]),
            ]
            w3 = [
                nc.sync.dma_start(out=wk_sb[:], in_=wkr),
                nc.gpsimd.dma_start(out=xb[:, 4, :], in_=xr[:, 4, :]),
                nc.scalar.dma_start(out=xb[:, 5, :], in_=xr[:, 5, :]),
            ]
            w4 = [
                nc.sync.dma_start(out=xb[:, 6, :], in_=xr[:, 6, :]),
                nc.gpsimd.dma_start(out=xb[:, 7, :], in_=xr[:, 7, :]),
                nc.scalar.dma_start(out=wo_sb[:], in_=wor),
            ]
            for prev, cur in ((w1, w2), (w2, w3), (w3, w4)):
                for dma in cur:
                    for dep in prev:
                        add_dep_helper(dma.ins, dep.ins, reason="dma wave order")

            q_sb = res.tile([P, NPAIR, L], BF16)
            k_sb = res.tile([P, NPAIR, L], BF16)
            vt_sb = res.tile([P, LT, 2 * NPAIR * HV], BF16)
            vt4 = vt_sb[:].rearrange("p l (h e) -> p l h e", e=HV)
            nc.vector.memset(vt4[:, :, :, DH : DH + 1], 1.0)
            c_sb = res.tile([P, NPAIR, L], F32)     # unnormalized C
            cn_sb = res.tile([P, NPAIR, L], BF16)   # normalized C

            # ---- upfront: pair-0 Q/K projections (kt-outer, ldweights
            # reuse across the 4 q-columns), full V projection ----
            # kt consumption order roughly matching waved DMA arrival
            KT_ORDER = (0, 1, 2, 3, 4, 5, 6, 7)

            # ---- sequential prefix (v4): Q pair-0, K pair-0 cols 0:1024,
            # V tiles 0..3 ----
            psA = ps_s.tile([P, 2 * QB], F32, tag="s")
            psB = ps_s.tile([P, 2 * QB], F32, tag="s")
            for ki, kt in enumerate(KT_ORDER):
                lhq = wq_sb[:, kt, 0:P]
                for half, ps in ((0, psA), (1, psB)):
                    for cb in range(2):
                        n0 = cb * QB
                        nc.tensor.matmul(
                            ps[:, n0 : n0 + QB],
                            lhsT=lhq,
                            rhs=xb[:, kt, half * 1024 + n0 : half * 1024 + n0 + QB],
                            start=(ki == 0),
                            stop=(ki == DC - 1),
                        )
                if ki < 5:
                    # warm-up: keep the HAM clock-gate at full rate while the
                    # projection is paced by the x DMA waves
                    dps = ps_f.tile([P, QB], F32, tag="f")
                    for _ in range(3):
                        nc.tensor.matmul(
                            dps[:], lhsT=xb[:, 0, 0:P], rhs=xb[:, 0, 0:QB],
                            start=True, stop=True,
                        )
            nc.vector.tensor_copy(out=q_sb[:, 0, 0:1024], in_=psA[:])
            nc.vector.tensor_copy(out=q_sb[:, 0, 1024:2048], in_=psB[:])

            psK = ps_s.tile([P, 2 * QB], F32, tag="s")
            for ki, kt in enumerate(KT_ORDER):
                for cb in range(2):
                    n0 = cb * QB
                    nc.tensor.matmul(
                        psK[:, n0 : n0 + QB],
                        lhsT=wk_sb[:, kt, 0:P],
                        rhs=xb[:, kt, n0 : n0 + QB],
                        start=(ki == 0),
                        stop=(ki == DC - 1),
                    )
            nc.vector.tensor_copy(out=k_sb[:, 0, 0:1024], in_=psK[:])

            def emit_vtile(lt, pool, tag):
                psv = pool.tile([P, 2 * NPAIR * DH], F32, tag=tag)
                for kt in range(DC):
                    nc.tensor.matmul(
                        psv[:],
                        lhsT=xb[:, kt, lt * P : (lt + 1) * P],
                        rhs=wv_sb[:, kt, :],
                        start=(kt == 0),
                        stop=(kt == DC - 1),
                    )
                nc.vector.tensor_copy(
                    out=vt4[:, lt, :, 0:DH],
                    in_=psv[:].rearrange("p (h e) -> p h e", e=DH),
                )

            for lt in range(4):
                emit_vtile(lt, ps_c, "c")

            # ---- filler generators (run inside attention units) ----
            def mk_proj_col(w_sb, dst, col, j=1):
                # one 512-wide column of a Q or K projection for pair j
                def f():
                    pc = ps_f.tile([P, QB], F32, tag="f")
                    for ki, kt in enumerate(KT_ORDER):
                        nc.tensor.matmul(
                            pc[:],
                            lhsT=w_sb[:, kt, j * P : (j + 1) * P],
                            rhs=xb[:, kt, col * QB : (col + 1) * QB],
                            start=(ki == 0),
                            stop=(ki == DC - 1),
                        )
                    nc.vector.tensor_copy(
                        out=dst[:, j, col * QB : (col + 1) * QB], in_=pc[:]
                    )
                return f

            def mk_outproj_mt(qb, mt, pool=None, ceng=None):
                # one 128-row block of the output projection for query block qb
                def f():
                    po = (pool or ps_f).tile(
                        [P, QB], F32, tag="f" if pool is None else "c"
                    )
                    q0 = qb * QB
                    for j in range(NPAIR):
                        nc.tensor.matmul(
                            po[:],
                            lhsT=wo_sb[:, j, mt * P : (mt + 1) * P],
                            rhs=cn_sb[:, j, q0 : q0 + QB],
                            start=(j == 0),
                            stop=(j == NPAIR - 1),
                        )
                    o_t = opool.tile([P, QB], BF16, tag="ot")
                    if ceng is nc.scalar:
                        nc.scalar.copy(o_t[:], po[:])
                    else:
                        nc.vector.tensor_copy(out=o_t[:], in_=po[:])
                    nc.sync.dma_start(out=outr[:, mt, q0 : q0 + QB], in_=o_t[:])
                return f

            # ---- attention units, software-pipelined across unit
            # boundaries: the next score pair is always emitted before the
            # current A@V so the ACT engine never drains its queue ----
            state = {}
            score_tiles = {}

            def emit_score(qb, j, t):
                q0 = qb * QB
                s = ps_s.tile([P, 2 * QB], F32, tag="s")
                nc.tensor.matmul(
                    s[:, 0:QB],
                    lhsT=k_sb[0:DH, j, t * P : (t + 1) * P],
                    rhs=q_sb[0:DH, j, q0 : q0 + QB],
                    start=True,
                    stop=True,
                )
                nc.tensor.matmul(
                    s[:, QB : 2 * QB],
                    lhsT=k_sb[DH:P, j, t * P : (t + 1) * P],
                    rhs=q_sb[DH:P, j, q0 : q0 + QB],
                    start=True,
                    stop=True,
                )
                score_tiles[(qb, j, t)] = s

            def emit_attention(qb, j, fillers=(), stride=4, next_first=None,
                               at6=None, late=()):
                c_a = ps_c.tile([HV, QB], F32, tag="c")
                c_b = ps_c.tile([HV, QB], F32, tag="c")
                fl = list(fillers)
                lt_fl = list(late)
                if (qb, j, 0) not in score_tiles:
                    emit_score(qb, j, 0)
                for t in range(LT):
                    s = score_tiles.pop((qb, j, t))
                    e = epool.tile([P, 2 * QB], BF16, tag="e")
                    nc.scalar.activation(e[:], s[:], AF.Exp, scale=float(SCALE))
                    if t < LT - 1:
                        emit_score(qb, j, t + 1)
                    elif next_first is not None:
                        emit_score(*next_first, 0)
                    nc.tensor.matmul(
                        c_a[:],
                        lhsT=vt4[:, t, 2 * j, :],
                        rhs=e[:, 0:QB],
                        start=(t == 0),
                        stop=(t == LT - 1),
                    )
                    nc.tensor.matmul(
                        c_b[:],
                        lhsT=vt4[:, t, 2 * j + 1, :],
                        rhs=e[:, QB : 2 * QB],
                        start=(t == 0),
                        stop=(t == LT - 1),
                    )
                    if t == 6 and at6 is not None:
                        at6()
                    if fl and t % stride == stride - 1:
                        fl.pop(0)()
                    if lt_fl and t >= 9:
                        lt_fl.pop(0)()
                for f in fl:
                    f()
                for f in lt_fl:
                    f()
                state[(qb, j)] = (c_a, c_b)

            def emit_norm_a(qb, j, den_first=False):
                # DVE-only: drain C, stage denominators, fast reciprocal.
                c_a, c_b = state[(qb, j)]
                q0 = qb * QB

                def drains():
                    nc.vector.tensor_copy(
                        out=c_sb[0:DH, j, q0 : q0 + QB], in_=c_a[0:DH, :]
                    )
                    nc.vector.tensor_copy(
                        out=c_sb[DH:P, j, q0 : q0 + QB], in_=c_b[0:DH, :]
                    )

                def den_stage():
                    den = npool.tile([2, QB], F32, tag="den")
                    nc.vector.tensor_copy(
                        out=den[0:1, :], in_=c_a[DH : DH + 1, :]
                    )
                    stage = npool.tile([1, QB], F32, tag="stg")
                    nc.vector.tensor_copy(out=stage[:], in_=c_b[DH : DH + 1, :])
                    nc.sync.dma_start(out=den[1:2, :], in_=stage[:])
                    return den

                def recip_of(den):
                    rf = npool.tile([2, QB], F32, tag="rf")
                    nc.vector.reciprocal_approx_fast(rf[:], den[:])
                    recip = npool.tile([2, QB], BF16, tag="rcp")
                    nc.vector.tensor_copy(out=recip[:], in_=rf[:])
                    state[(qb, j, "r")] = recip

                if den_first:
                    den = den_stage()
                    drains()
                    recip_of(den)
                else:
                    drains()
                    recip_of(den_stage())

            selp_sb = consts.tile([2, P], BF16)
            nc.vector.tensor_copy(out=selp_sb[0:1, :], in_=sel2_sb[0:1, :])
            stage_m = consts.tile([1, P], BF16)
            nc.vector.tensor_copy(out=stage_m[:], in_=sel2_sb[DH : DH + 1, :])
            nc.sync.dma_start(out=selp_sb[1:2, :], in_=stage_m[:])

            def emit_norm_b(qb, j):
                recip = state.pop((qb, j, "r"))
                state.pop((qb, j))
                q0 = qb * QB
                bc = ps_c.tile([P, QB], F32, tag="c")
                nc.tensor.matmul(
                    bc[:], lhsT=selp_sb[:], rhs=recip[:], start=True, stop=True
                )
                nc.vector.tensor_mul(
                    out=cn_sb[:, j, q0 : q0 + QB],
                    in0=c_sb[:, j, q0 : q0 + QB],
                    in1=bc[:],
                )

            # unit order: pair-1 enters after 3 pair-0 units so its Q/K
            # projections (fillers in units 1-2) are done; out-proj for a
            # query block fills a later unit once both pairs are normalized;
            # only qb3's out-proj remains for the tail.
            units = [
                (0, 0), (1, 0), (2, 0), (0, 1),
                (1, 1), (2, 1), (3, 0), (3, 1),
            ]
            cols = [mk_proj_col(wq_sb, q_sb, c) for c in range(NQB)] + [
                mk_proj_col(wk_sb, k_sb, c) for c in range(NQB)
            ]
            fillers_by_idx = {
                0: [mk_proj_col(wk_sb, k_sb, 2, j=0),
                    mk_proj_col(wk_sb, k_sb, 3, j=0)]
                   + [(lambda lt: (lambda: emit_vtile(lt, ps_f, "f")))(lt)
                      for lt in range(4, LT)],
                1: cols[0:4],
                2: cols[4:8],
            }
            strides = {0: 1, 1: 4, 2: 4}
            # norm_b for unit u runs as a t=6 hook inside unit u+1 (its
            # reciprocal is ready by then); out-proj for query block qb runs
            # as late fillers (t>=9) of the unit where (qb, pair1)'s norm_b
            # fires, so only qb3's out-proj is left for the tail.
            at6_by_idx = {
                idx + 1: (lambda u: (lambda: emit_norm_b(*u)))(units[idx])
                for idx in range(len(units) - 1)
            }
            late_by_idx = {
                4: [mk_outproj_mt(0, mt) for mt in range(DC)],
                5: [mk_outproj_mt(1, mt) for mt in range(DC)],
                6: [mk_outproj_mt(2, mt) for mt in range(DC)],
            }
            for idx, (qb, j) in enumerate(units):
                emit_attention(
                    qb, j,
                    fillers_by_idx.get(idx, ()),
                    strides.get(idx, 4),
                    next_first=units[idx + 1] if idx + 1 < len(units) else None,
                    at6=at6_by_idx.get(idx),
                    late=late_by_idx.get(idx, ()),
                )
                emit_norm_a(qb, j, den_first=(idx == len(units) - 1))
            # tail: keep the PE warm through the final norm chain, then the
            # last query block's out-proj.
            for i in range(10):
                dps = ps_f.tile([P, QB], F32, tag="f")
                nc.tensor.matmul(
                    dps[:], lhsT=xb[:, 0, 0:P], rhs=xb[:, 0, 0:QB],
                    start=True, stop=True,
                )
            emit_norm_b(*units[-1])
            for mt in range(DC):
                mk_outproj_mt(
                    3, mt, pool=ps_c if mt % 2 else None,
                    ceng=nc.scalar if mt % 2 else nc.vector,
                )()

    if not nc.is_finalized():
        nc.finalize()
    return nc


_NC_CACHE = {}


def _get_nc():
    if "nc" not in _NC_CACHE:
        _NC_CACHE["nc"] = build()
    return _NC_CACHE["nc"]


def _run(x, Wq, Wk, Wv, Wo, trace=False):
    """x: (B, D, L) f32; W*: (D, D) f32. Returns (out, BassKernelResults)."""
    nc = _get_nc()
    bf = ml_dtypes.bfloat16
    xb = np.ascontiguousarray(x).astype(bf)                 # (B, D, L)
    wqt = np.ascontiguousarray(np.asarray(Wq, np.float32).T).astype(bf)
    wkt = np.ascontiguousarray(np.asarray(Wk, np.float32).T).astype(bf)
    wvt = np.ascontiguousarray(np.asarray(Wv, np.float32).T).astype(bf)
    wot = np.ascontiguousarray(np.asarray(Wo, np.float32).T).astype(bf)

    sel2 = np.zeros((DH + 1, P), np.float32)
    sel2[0, 0:DH] = 1.0
    sel2[DH, DH:P] = 1.0
    sel2 = sel2.astype(bf)

    in_maps = []
    for c in range(8):
        b = c // 4
        g = c % 4
        r0 = g * NPAIR * P
        in_maps.append(
            {
                "x": xb[b],
                "wq": np.ascontiguousarray(wqt[:, r0 : r0 + NPAIR * P]),
                "wk": np.ascontiguousarray(wkt[:, r0 : r0 + NPAIR * P]),
                "wv": np.ascontiguousarray(wvt[:, r0 : r0 + NPAIR * P]),
                "wo": np.ascontiguousarray(wot[r0 : r0 + NPAIR * P, :]),
                "sel2": sel2,
            }
        )
    res = run_bass_kernel_spmd(nc, in_maps, core_ids=list(range(8)), trace=trace)
    out = np.zeros((B, D, L), np.float32)
    for c in range(8):
        b = c // 4
        out[b] += res.results[c]["out"].astype(np.float32)
    return out, res


def kernel(x, mask, Wq, Wk, Wv, Wo):
    # mask is all-ones by construction (fill: ones) -- softmax over all keys.
    out, _ = _run(x, Wq, Wk, Wv, Wo, trace=False)
    return out


# revision 47
# speedup vs baseline: 1.0194x; 1.0194x over previous
"""Multi-head attention (B=2, D=1024, L=2048, H=16) on 8 TRN2 NeuronCores.

Sharding (per spec hint): tensor-parallel over heads x data-parallel over
batch.  Core c handles batch c//4 and head group g=c%4 (4 heads as 2
"pairs" of 2 heads stacked on 64-partition halves).  Host sums the 4
bf16 partial outputs per batch (row-parallel W_O) in f32 -- no on-device
collective.

Per core:
  - Q/K projected into pair layout (head dims on partitions); V projected
    directly into transposed layout VT (Lk x dh) with a ones column per
    head so A@V also emits the softmax denominator.
  - Scores ST[k,q]: the two heads' K=64 matmuls are issued back-to-back
    at partition bases 0/64; they run concurrently on disjoint PE row
    groups (~2x, verified in trace).
  - exp runs on the scalar/ACT engine (the only exp engine): 128 x
    (128,1024) activations ~= 142us is the kernel's pacing floor.  All
    other PE work (pair-1 Q/K projections, output projection) is emitted
    as fillers INSIDE attention units so it executes in the PE's
    ACT-bound gaps instead of serializing.
  - Normalization per (pair, qb): denominator rows -> DVE reciprocal ->
    K=2 selector matmul broadcast -> one multiply.  The matmul+multiply
    (norm_b) is emitted one unit later so the PE queue never waits on
    the (slow, ~3.3us) DVE reciprocal.

All matmuls bf16 (f32 PSUM accumulate); softmax stats f32.
"""

import sys
import types

import numpy as np
import ml_dtypes


def _install_axon_hooks_shim():
    try:
        import antenv.axon_hooks  # noqa: F401
        return
    except ImportError:
        pass
    try:
        import antenv
    except ImportError:
        return
    mod = types.ModuleType("antenv.axon_hooks")
    mod._hook = None
    mod.set_axon_ntff_profile_hook = lambda h: setattr(mod, "_hook", h)
    mod.get_axon_ntff_profile_hook = lambda: mod._hook
    sys.modules["antenv.axon_hooks"] = mod
    antenv.axon_hooks = mod
    try:
        from trn_agent_boot.trn_boot import _ntff_profile_via_ctypes

        h = _ntff_profile_via_ctypes("/opt/axon/libaxon_pjrt.so")
        if h is not None:
            mod._hook = h
    except Exception:
        pass


_install_axon_hooks_shim()

import concourse.bass as bass  # noqa: E402
import concourse.mybir as mybir  # noqa: E402
import concourse.tile as tile  # noqa: E402
from concourse import bacc  # noqa: E402
from concourse.bass_utils import run_bass_kernel_spmd  # noqa: E402
from concourse.tile_rust import add_dep_helper  # noqa: E402

BF16 = mybir.dt.bfloat16
F32 = mybir.dt.float32
AF = mybir.ActivationFunctionType

B, D, L, H = 2, 1024, 2048, 16
DH = D // H            # 64
P = 128
SCALE = 1.0 / np.sqrt(np.float32(DH))

DC = D // P            # 8 contraction chunks over D
LT = L // P            # 16 Lk tiles
HV = DH + 1            # V^T per-head width incl. ones column
NPAIR = 2              # head pairs per core (4 heads)
NQB = 4                # query blocks of 512
QB = L // NQB          # 512


def build():
    nc = bacc.Bacc(None, target_bir_lowering=False, debug=False)

    x = nc.dram_tensor("x", [D, L], BF16, kind="ExternalInput")
    wq = nc.dram_tensor("wq", [D, NPAIR * P], BF16, kind="ExternalInput")
    wk = nc.dram_tensor("wk", [D, NPAIR * P], BF16, kind="ExternalInput")
    wv = nc.dram_tensor("wv", [D, NPAIR * P], BF16, kind="ExternalInput")
    wo = nc.dram_tensor("wo", [NPAIR * P, D], BF16, kind="ExternalInput")
    sel2 = nc.dram_tensor("sel2", [DH + 1, P], BF16, kind="ExternalInput")
    out = nc.dram_tensor("out", [D, L], BF16, kind="ExternalOutput")

    xr = x[:].rearrange("(o p) l -> p o l", p=P)          # (128, 8, 2048)
    wqr = wq[:].rearrange("(o p) m -> p o m", p=P)        # (128, 8, 256)
    wkr = wk[:].rearrange("(o p) m -> p o m", p=P)
    wvr = wv[:].rearrange("(o p) m -> p o m", p=P)
    wor = wo[:].rearrange("(j p) o -> p j o", p=P)        # (128, 2, 1024)
    outr = out[:].rearrange("(o p) l -> p o l", p=P)      # (128, 8, 2048)

    with tile.TileContext(nc) as tc:
        with (
            tc.tile_pool(name="consts", bufs=1) as consts,
            tc.tile_pool(name="res", bufs=1) as res,
            tc.tile_pool(name="exp", bufs=3) as epool,
            tc.tile_pool(name="norm", bufs=2) as npool,
            tc.tile_pool(name="outp", bufs=3) as opool,
            tc.tile_pool(name="ps_s", bufs=2, space="PSUM") as ps_s,
            tc.tile_pool(name="ps_c", bufs=3, space="PSUM") as ps_c,
            tc.tile_pool(name="ps_f", bufs=1, space="PSUM") as ps_f,
        ):
            # DMA wave sequencing: the DMA engines round-robin among ALL
            # in-flight transfers, so if everything is enqueued at once the
            # startup-critical wq/x0/x1 only land when ~everything lands
            # (~24us).  Explicit wave dependencies keep at most 3-4
            # transfers in flight so the early waves get full bandwidth.
            sel2_sb = consts.tile([DH + 1, P], BF16)
            wq_sb = res.tile([P, DC, NPAIR * P], BF16)
            wk_sb = res.tile([P, DC, NPAIR * P], BF16)
            wv_sb = res.tile([P, DC, NPAIR * P], BF16)
            wo_sb = res.tile([P, NPAIR, D], BF16)
            xb = res.tile([P, DC, L], BF16)

            nc.sync.dma_start(out=sel2_sb[:], in_=sel2[:])
            w1 = [
                nc.sync.dma_start(out=wq_sb[:], in_=wqr),
                nc.gpsimd.dma_start(out=xb[:, 0, :], in_=xr[:, 0, :]),
                nc.scalar.dma_start(out=wv_sb[:], in_=wvr),
            ]
            w2 = [
                nc.sync.dma_start(out=xb[:, 1, :], in_=xr[:, 1, :]),
                nc.gpsimd.dma_start(out=xb[:, 2, :], in_=xr[:, 2, :]),
                nc.scalar.dma_start(out=xb[:, 3, :], in_=xr[:, 3, :]),
            ]
            w3 = [
                nc.sync.dma_start(out=wk_sb[:], in_=wkr),
                nc.gpsimd.dma_start(out=xb[:, 4, :], in_=xr[:, 4, :]),
                nc.scalar.dma_start(out=xb[:, 5, :], in_=xr[:, 5, :]),
            ]
            w4 = [
                nc.sync.dma_start(out=xb[:, 6, :], in_=xr[:, 6, :]),
                nc.gpsimd.dma_start(out=xb[:, 7, :], in_=xr[:, 7, :]),
                nc.scalar.dma_start(out=wo_sb[:], in_=wor),
            ]
            for prev, cur in ((w1, w2), (w2, w3), (w3, w4)):
                for dma in cur:
                    for dep in prev:
                        add_dep_helper(dma.ins, dep.ins, reason="dma wave order")

            q_sb = res.tile([P, NPAIR, L], BF16)
            k_sb = res.tile([P, NPAIR, L], BF16)
            vt_sb = res.tile([P, LT, 2 * NPAIR * HV], BF16)
            vt4 = vt_sb[:].rearrange("p l (h e) -> p l h e", e=HV)
            nc.vector.memset(vt4[:, :, :, DH : DH + 1], 1.0)
            c_sb = res.tile([P, NPAIR, L], F32)     # unnormalized C
            cn_sb = res.tile([P, NPAIR, L], BF16)   # normalized C

            # ---- upfront: pair-0 Q/K projections (kt-outer, ldweights
            # reuse across the 4 q-columns), full V projection ----
            # kt consumption order roughly matching waved DMA arrival
            KT_ORDER = (0, 1, 2, 3, 4, 5, 6, 7)

            # ---- sequential prefix (v4): Q pair-0, K pair-0 cols 0:1024,
            # V tiles 0..3 ----
            psA = ps_s.tile([P, 2 * QB], F32, tag="s")
            psB = ps_s.tile([P, 2 * QB], F32, tag="s")
            for ki, kt in enumerate(KT_ORDER):
                lhq = wq_sb[:, kt, 0:P]
                for half, ps in ((0, psA), (1, psB)):
                    for cb in range(2):
                        n0 = cb * QB
                        nc.tensor.matmul(
                            ps[:, n0 : n0 + QB],
                            lhsT=lhq,
                            rhs=xb[:, kt, half * 1024 + n0 : half * 1024 + n0 + QB],
                            start=(ki == 0),
                            stop=(ki == DC - 1),
                        )
                if ki < 5:
                    # warm-up: keep the HAM clock-gate at full rate while the
                    # projection is paced by the x DMA waves
                    dps = ps_f.tile([P, QB], F32, tag="f")
                    for _ in range(3):
                        nc.tensor.matmul(
                            dps[:], lhsT=xb[:, 0, 0:P], rhs=xb[:, 0, 0:QB],
                            start=True, stop=True,
                        )
            nc.vector.tensor_copy(out=q_sb[:, 0, 0:1024], in_=psA[:])
            nc.vector.tensor_copy(out=q_sb[:, 0, 1024:2048], in_=psB[:])

            # K pair-0 cols 0:1024 in its own PSUM pool so it starts without
            # waiting for the Q drains (a PE gap there triggers a >=3.4us
            # HAM half-clock window)
            psK0 = ps_c.tile([P, QB], F32, tag="c")
            psK1 = ps_c.tile([P, QB], F32, tag="c")
            for ki, kt in enumerate(KT_ORDER):
                for psk, n0 in ((psK0, 0), (psK1, QB)):
                    nc.tensor.matmul(
                        psk[:],
                        lhsT=wk_sb[:, kt, 0:P],
                        rhs=xb[:, kt, n0 : n0 + QB],
                        start=(ki == 0),
                        stop=(ki == DC - 1),
                    )
            nc.vector.tensor_copy(out=k_sb[:, 0, 0:QB], in_=psK0[:])
            nc.vector.tensor_copy(out=k_sb[:, 0, QB : 2 * QB], in_=psK1[:])

            def emit_vtile(lt, pool, tag):
                psv = pool.tile([P, 2 * NPAIR * DH], F32, tag=tag)
                for kt in range(DC):
                    nc.tensor.matmul(
                        psv[:],
                        lhsT=xb[:, kt, lt * P : (lt + 1) * P],
                        rhs=wv_sb[:, kt, :],
                        start=(kt == 0),
                        stop=(kt == DC - 1),
                    )
                nc.vector.tensor_copy(
                    out=vt4[:, lt, :, 0:DH],
                    in_=psv[:].rearrange("p (h e) -> p h e", e=DH),
                )

            for _ in range(3):
                dps = ps_f.tile([P, QB], F32, tag="f")
                nc.tensor.matmul(
                    dps[:], lhsT=xb[:, 0, 0:P], rhs=xb[:, 0, 0:QB],
                    start=True, stop=True,
                )
            for lt in range(4):
                emit_vtile(lt, ps_c, "c")

            # ---- filler generators (run inside attention units) ----
            def mk_proj_col(w_sb, dst, col, j=1):
                # one 512-wide column of a Q or K projection for pair j
                def f():
                    pc = ps_f.tile([P, QB], F32, tag="f")
                    for ki, kt in enumerate(KT_ORDER):
                        nc.tensor.matmul(
                            pc[:],
                            lhsT=w_sb[:, kt, j * P : (j + 1) * P],
                            rhs=xb[:, kt, col * QB : (col + 1) * QB],
                            start=(ki == 0),
                            stop=(ki == DC - 1),
                        )
                    nc.vector.tensor_copy(
                        out=dst[:, j, col * QB : (col + 1) * QB], in_=pc[:]
                    )
                return f

            def mk_outproj_mt(qb, mt, pool=None, ceng=None):
                # one 128-row block of the output projection for query block qb
                def f():
                    po = (pool or ps_f).tile(
                        [P, QB], F32, tag="f" if pool is None else "c"
                    )
                    q0 = qb * QB
                    for j in range(NPAIR):
                        nc.tensor.matmul(
                            po[:],
                            lhsT=wo_sb[:, j, mt * P : (mt + 1) * P],
                            rhs=cn_sb[:, j, q0 : q0 + QB],
                            start=(j == 0),
                            stop=(j == NPAIR - 1),
                        )
                    o_t = opool.tile([P, QB], BF16, tag="ot")
                    if ceng is nc.scalar:
                        nc.scalar.copy(o_t[:], po[:])
                    else:
                        nc.vector.tensor_copy(out=o_t[:], in_=po[:])
                    nc.sync.dma_start(out=outr[:, mt, q0 : q0 + QB], in_=o_t[:])
                return f

            # ---- attention units, software-pipelined across unit
            # boundaries: the next score pair is always emitted before the
            # current A@V so the ACT engine never drains its queue ----
            state = {}
            score_tiles = {}

            def emit_score(qb, j, t):
                q0 = qb * QB
                s = ps_s.tile([P, 2 * QB], F32, tag="s")
                nc.tensor.matmul(
                    s[:, 0:QB],
                    lhsT=k_sb[0:DH, j, t * P : (t + 1) * P],
                    rhs=q_sb[0:DH, j, q0 : q0 + QB],
                    start=True,
                    stop=True,
                )
                nc.tensor.matmul(
                    s[:, QB : 2 * QB],
                    lhsT=k_sb[DH:P, j, t * P : (t + 1) * P],
                    rhs=q_sb[DH:P, j, q0 : q0 + QB],
                    start=True,
                    stop=True,
                )
                score_tiles[(qb, j, t)] = s

            def emit_attention(qb, j, fillers=(), stride=4, next_first=None,
                               at6=None, late=()):
                c_a = ps_c.tile([HV, QB], F32, tag="c")
                c_b = ps_c.tile([HV, QB], F32, tag="c")
                fl = list(fillers)
                lt_fl = list(late)
                if (qb, j, 0) not in score_tiles:
                    emit_score(qb, j, 0)
                for t in range(LT):
                    s = score_tiles.pop((qb, j, t))
                    e = epool.tile([P, 2 * QB], BF16, tag="e")
                    nc.scalar.activation(e[:], s[:], AF.Exp, scale=float(SCALE))
                    if t < LT - 1:
                        if (qb, j, t + 1) not in score_tiles:
                            emit_score(qb, j, t + 1)
                        if t == LT - 2 and next_first is not None:
                            emit_score(*next_first, 0)
                    elif next_first is not None:
                        emit_score(*next_first, 1)
                    nc.tensor.matmul(
                        c_a[:],
                        lhsT=vt4[:, t, 2 * j, :],
                        rhs=e[:, 0:QB],
                        start=(t == 0),
                        stop=(t == LT - 1),
                    )
                    nc.tensor.matmul(
                        c_b[:],
                        lhsT=vt4[:, t, 2 * j + 1, :],
                        rhs=e[:, QB : 2 * QB],
                        start=(t == 0),
                        stop=(t == LT - 1),
                    )
                    if t == 6 and at6 is not None:
                        at6()
                    if fl and t % stride == stride - 1:
                        fl.pop(0)()
                    if lt_fl and t >= 9:
                        lt_fl.pop(0)()
                for f in fl:
                    f()
                for f in lt_fl:
                    f()
                state[(qb, j)] = (c_a, c_b)

            def emit_norm_a(qb, j, den_first=False):
                # DVE-only: drain C, stage denominators, fast reciprocal.
                c_a, c_b = state[(qb, j)]
                q0 = qb * QB

                def drains():
                    nc.vector.tensor_copy(
                        out=c_sb[0:DH, j, q0 : q0 + QB], in_=c_a[0:DH, :]
                    )
                    nc.vector.tensor_copy(
                        out=c_sb[DH:P, j, q0 : q0 + QB], in_=c_b[0:DH, :]
                    )

                def den_stage():
                    den = npool.tile([2, QB], F32, tag="den")
                    nc.vector.tensor_copy(
                        out=den[0:1, :], in_=c_a[DH : DH + 1, :]
                    )
                    stage = npool.tile([1, QB], F32, tag="stg")
                    nc.vector.tensor_copy(out=stage[:], in_=c_b[DH : DH + 1, :])
                    nc.sync.dma_start(out=den[1:2, :], in_=stage[:])
                    return den

                def recip_of(den):
                    rf = npool.tile([2, QB], F32, tag="rf")
                    nc.vector.reciprocal_approx_fast(rf[:], den[:])
                    recip = npool.tile([2, QB], BF16, tag="rcp")
                    nc.vector.tensor_copy(out=recip[:], in_=rf[:])
                    state[(qb, j, "r")] = recip

                if den_first:
                    den = den_stage()
                    drains()
                    recip_of(den)
                else:
                    drains()
                    recip_of(den_stage())

            selp_sb = consts.tile([2, P], BF16)
            nc.vector.tensor_copy(out=selp_sb[0:1, :], in_=sel2_sb[0:1, :])
            stage_m = consts.tile([1, P], BF16)
            nc.vector.tensor_copy(out=stage_m[:], in_=sel2_sb[DH : DH + 1, :])
            nc.sync.dma_start(out=selp_sb[1:2, :], in_=stage_m[:])

            def emit_norm_b(qb, j):
                recip = state.pop((qb, j, "r"))
                state.pop((qb, j))
                q0 = qb * QB
                bc = ps_c.tile([P, QB], F32, tag="c")
                nc.tensor.matmul(
                    bc[:], lhsT=selp_sb[:], rhs=recip[:], start=True, stop=True
                )
                nc.vector.tensor_mul(
                    out=cn_sb[:, j, q0 : q0 + QB],
                    in0=c_sb[:, j, q0 : q0 + QB],
                    in1=bc[:],
                )

            # unit order: pair-1 enters after 3 pair-0 units so its Q/K
            # projections (fillers in units 1-2) are done; out-proj for a
            # query block fills a later unit once both pairs are normalized;
            # only qb3's out-proj remains for the tail.
            units = [
                (0, 0), (1, 0), (2, 0), (0, 1),
                (1, 1), (2, 1), (3, 0), (3, 1),
            ]
            cols = [mk_proj_col(wq_sb, q_sb, c) for c in range(NQB)] + [
                mk_proj_col(wk_sb, k_sb, c) for c in range(NQB)
            ]
            fillers_by_idx = {
                0: [mk_proj_col(wk_sb, k_sb, 2, j=0),
                    mk_proj_col(wk_sb, k_sb, 3, j=0)]
                   + [(lambda lt: (lambda: emit_vtile(lt, ps_f, "f")))(lt)
                      for lt in range(4, LT)],
                1: cols[0:4],
                2: cols[4:8],
            }
            strides = {0: 1, 1: 4, 2: 4}
            # norm_b for unit u runs as a t=6 hook inside unit u+1 (its
            # reciprocal is ready by then); out-proj for query block qb runs
            # as late fillers (t>=9) of the unit where (qb, pair1)'s norm_b
            # fires, so only qb3's out-proj is left for the tail.
            at6_by_idx = {
                idx + 1: (lambda u: (lambda: emit_norm_b(*u)))(units[idx])
                for idx in range(len(units) - 1)
            }
            late_by_idx = {
                4: [mk_outproj_mt(0, mt) for mt in range(DC)],
                5: [mk_outproj_mt(1, mt) for mt in range(DC)],
                6: [mk_outproj_mt(2, mt) for mt in range(DC)],
            }
            for idx, (qb, j) in enumerate(units):
                emit_attention(
                    qb, j,
                    fillers_by_idx.get(idx, ()),
                    strides.get(idx, 4),
                    next_first=units[idx + 1] if idx + 1 < len(units) else None,
                    at6=at6_by_idx.get(idx),
                    late=late_by_idx.get(idx, ()),
                )
                emit_norm_a(qb, j, den_first=(idx == len(units) - 1))
            # tail: keep the PE warm through the final norm chain, then the
            # last query block's out-proj.
            for i in range(20):
                dps = ps_f.tile([P, QB], F32, tag="f")
                nc.tensor.matmul(
                    dps[:], lhsT=xb[:, 0, 0:P], rhs=xb[:, 0, 0:QB],
                    start=True, stop=True,
                )
            emit_norm_b(*units[-1])
            for i in range(4):
                dps = ps_f.tile([P, QB], F32, tag="f")
                nc.tensor.matmul(
                    dps[:], lhsT=xb[:, 0, 0:P], rhs=xb[:, 0, 0:QB],
                    start=True, stop=True,
                )
            for mt in range(DC):
                mk_outproj_mt(
                    3, mt, pool=ps_c if mt % 2 else None,
                    ceng=nc.scalar if mt % 2 else nc.vector,
                )()

    if not nc.is_finalized():
        nc.finalize()
    return nc


_NC_CACHE = {}


def _get_nc():
    if "nc" not in _NC_CACHE:
        _NC_CACHE["nc"] = build()
    return _NC_CACHE["nc"]


def _run(x, Wq, Wk, Wv, Wo, trace=False):
    """x: (B, D, L) f32; W*: (D, D) f32. Returns (out, BassKernelResults)."""
    nc = _get_nc()
    bf = ml_dtypes.bfloat16
    xb = np.ascontiguousarray(x).astype(bf)                 # (B, D, L)
    wqt = np.ascontiguousarray(np.asarray(Wq, np.float32).T).astype(bf)
    wkt = np.ascontiguousarray(np.asarray(Wk, np.float32).T).astype(bf)
    wvt = np.ascontiguousarray(np.asarray(Wv, np.float32).T).astype(bf)
    wot = np.ascontiguousarray(np.asarray(Wo, np.float32).T).astype(bf)

    sel2 = np.zeros((DH + 1, P), np.float32)
    sel2[0, 0:DH] = 1.0
    sel2[DH, DH:P] = 1.0
    sel2 = sel2.astype(bf)

    in_maps = []
    for c in range(8):
        b = c // 4
        g = c % 4
        r0 = g * NPAIR * P
        in_maps.append(
            {
                "x": xb[b],
                "wq": np.ascontiguousarray(wqt[:, r0 : r0 + NPAIR * P]),
                "wk": np.ascontiguousarray(wkt[:, r0 : r0 + NPAIR * P]),
                "wv": np.ascontiguousarray(wvt[:, r0 : r0 + NPAIR * P]),
                "wo": np.ascontiguousarray(wot[r0 : r0 + NPAIR * P, :]),
                "sel2": sel2,
            }
        )
    res = run_bass_kernel_spmd(nc, in_maps, core_ids=list(range(8)), trace=trace)
    out = np.zeros((B, D, L), np.float32)
    for c in range(8):
        b = c // 4
        out[b] += res.results[c]["out"].astype(np.float32)
    return out, res


def kernel(x, mask, Wq, Wk, Wv, Wo):
    # mask is all-ones by construction (fill: ones) -- softmax over all keys.
    out, _ = _run(x, Wq, Wk, Wv, Wo, trace=False)
    return out
